# revision 9
# baseline (speedup 1.0000x reference)
"""Trainium2 Bass kernel for nn_Attention_4380866642117.

Math: the reference computes additive-score attention
    score[b,i,j] = q[b,i].w_q + k[b,j].w_k ; masked (mask==True -> -1e10)
    attn = softmax_j(score); out = LN(attn @ v @ fc_w.T + q)
Because the score is additive, the q-term is constant along the softmax axis
and cancels; masked logits (-1e10) underflow to exactly 0 in the f32 softmax.
Hence with e_j = exp(k[b,j].w_k):
    attn[b,i,j] = (1-m[b,i,j]) * e_j / Z_i,  Z_i = sum_j (1-m) e_j
    out_pre_ln[b,i,:] = (1/Z_i) * sum_j (1-m) e_j (v[b,j,:] @ fc_w.T) + q
so the whole attention+fc collapses to one masked matmul with fc-transformed,
e-weighted values, plus a rank-1 elementwise product for the attn output.

Distribution: data-parallel over batch (8 batches -> 8 NeuronCores).

The mask must enter the PE with j on partitions (contraction dim).  The u8
mask is loaded a second time through the DMA xbar transpose by viewing pairs
of mask bytes as one fp16 element: the transposed SBUF tile then holds rows
j=2p and j=2p+1 interleaved on partition p.  The resulting j-permutation
sigma(c,p,b) = 256c + 2p + b is absorbed by loading k and v rows with the
same permuted access pattern; the contraction is order-invariant.
"""

import numpy as np

import concourse.bass as bass
import concourse.tile as tile
from concourse import mybir
from concourse.bass_utils import run_bass_kernel_spmd

F32 = mybir.dt.float32
F16 = mybir.dt.float16
U8 = mybir.dt.uint8
AF = mybir.ActivationFunctionType
ALU = mybir.AluOpType

B, LQ, LK, D = 8, 2048, 2048, 256
DEBUG_OUTPUTS = False
NT = LK // 128  # 16 j-tiles (and i-tiles)
LN_EPS = 1e-5

# ---------------------------------------------------------------------------
# walrus in this container supports at most ONE sync wait per instruction;
# Tile emits several.  Hoist excess waits onto same-engine NoOps.
# ---------------------------------------------------------------------------


def _fix_sync_waits(nc, max_waits: int = 1) -> int:
    n = 0
    for f in nc.m.functions:
        for bb in f.blocks:
            out = []
            changed = False
            for inst in bb.instructions:
                si = inst.sync_info
                waits = list(si.on_wait) if si is not None else []
                if len(waits) > max_waits and inst.engine is not None:
                    changed = True
                    rest = waits[max_waits:]
                    for i in range(0, len(rest), max_waits):
                        nop = mybir.InstNoOp(
                            name=f"{inst.name}-syncw{n}",
                            sync_info=mybir.SyncInfo(
                                on_wait=rest[i : i + max_waits], on_update=[]
                            ),
                            bass_nofuse=True,
                            engine=inst.engine,
                        )
                        n += 1
                        out.append(nop)
                    inst.sync_info = mybir.SyncInfo(
                        on_wait=waits[:max_waits], on_update=list(si.on_update)
                    )
                out.append(inst)
            if changed:
                bb.instructions = out
    return n


def _broadcast_load(nc, sbuf_tile, dram_ap, parts=128):
    """DMA a [1, N] DRAM row replicated across `parts` partitions."""
    src = bass.AP(
        tensor=dram_ap.tensor,
        offset=dram_ap.offset,
        ap=[[0, parts]] + list(dram_ap.ap[1:]),
    )
    nc.gpsimd.dma_start(out=sbuf_tile, in_=src)


def build_nc():
    nc = bass.Bass("TRN2")

    q_d = nc.dram_tensor("q", [LQ, D], F32, kind="ExternalInput")
    k_d = nc.dram_tensor("k", [LK, D], F32, kind="ExternalInput")
    v_d = nc.dram_tensor("v", [LK, D], F32, kind="ExternalInput")
    mask_d = nc.dram_tensor("mask", [LQ, LK], U8, kind="ExternalInput")
    wk_d = nc.dram_tensor("wk", [1, D], F32, kind="ExternalInput")
    fcw_d = nc.dram_tensor("fcw", [D, D], F32, kind="ExternalInput")
    gamma_d = nc.dram_tensor("gamma", [1, D], F32, kind="ExternalInput")
    beta_d = nc.dram_tensor("beta", [1, D], F32, kind="ExternalInput")
    ident_d = nc.dram_tensor("ident", [128, 128], F32, kind="ExternalInput")
    pb_d = nc.dram_tensor("pb", [128, 2, 256], F32, kind="ExternalInput")

    out_d = nc.dram_tensor("out", [LQ, D], F32, kind="ExternalOutput")
    attn_d = nc.dram_tensor("attn", [LQ, LK], F32, kind="ExternalOutput")
    if DEBUG_OUTPUTS:
        y_dbg = nc.dram_tensor("y_dbg", [LQ, D], F32, kind="ExternalOutput")
        x2_dbg = nc.dram_tensor("x2_dbg", [LQ, D], F32, kind="ExternalOutput")

    # permuted row views: j = 256c + 2p + b  ->  tile t = 2c + b, partition p
    k_perm = k_d[:, :].rearrange("(c p two) d -> c two p d", p=128, two=2)
    v_perm = v_d[:, :].rearrange("(c p two) d -> c two p d", p=128, two=2)
    # mask viewed as fp16 pairs for the xbar transpose
    mask_f16 = mask_d[:, :].bitcast(F16)  # [2048, 1024]

    with tile.TileContext(nc) as tc:
        with (
            tc.tile_pool(name="const", bufs=1) as const,
            tc.tile_pool(name="stat", bufs=1) as stat,
            tc.tile_pool(name="kv", bufs=3) as kv,
            tc.tile_pool(name="dump", bufs=2) as dumpp,
            tc.tile_pool(name="mt", bufs=2) as mtp,
            tc.tile_pool(name="big", bufs=2) as big,
            tc.tile_pool(name="attnp", bufs=3) as attnp,
            tc.tile_pool(name="mnat", bufs=3) as mnatp,
            tc.tile_pool(name="small", bufs=3) as small,
            tc.tile_pool(name="psm", bufs=2, space="PSUM") as psm,
            tc.tile_pool(name="psB", bufs=2, space="PSUM") as psB,
            tc.tile_pool(name="psN", bufs=2, space="PSUM") as psN,
        ):
            # ---------------- constants ----------------
            ident = const.tile([128, 128], F32)
            nc.sync.dma_start(out=ident, in_=ident_d[:, :])
            pb = const.tile([128, 2, 256], F32)
            nc.sync.dma_start(out=pb, in_=pb_d[:, :, :])
            wk_bc = const.tile([128, D], F32)
            _broadcast_load(nc, wk_bc, wk_d[0:1, :])
            gamma_bc = const.tile([128, D], F32)
            _broadcast_load(nc, gamma_bc, gamma_d[0:1, :])
            beta_bc = const.tile([128, D], F32)
            _broadcast_load(nc, beta_bc, beta_d[0:1, :])
            ones_row = const.tile([1, 128], F32)
            nc.vector.memset(ones_row, 1.0)
            eps_sb = const.tile([128, 1], F32)
            nc.vector.memset(eps_sb, LN_EPS)

            # fcwT[d, c] tiles from fcw[c, d]
            fcw_sb = const.tile([128, 2, 256], F32)
            nc.sync.dma_start(
                out=fcw_sb, in_=fcw_d[:, :].rearrange("(t p) d -> p t d", p=128)
            )
            fcwT = const.tile([128, 2, 256], F32)
            for dt in range(2):
                ps_t = psm.tile([128, 512], F32, tag="ps")
                for ct in range(2):
                    nc.tensor.transpose(
                        ps_t[:, 128 * ct : 128 * (ct + 1)],
                        fcw_sb[:, ct, 128 * dt : 128 * (dt + 1)],
                        ident,
                    )
                nc.vector.tensor_copy(fcwT[:, dt, :], ps_t[:, 0:256])

            # ---------------- persistent activations ----------------
            sk_sb = stat.tile([128, NT], F32)
            e_perm = stat.tile([128, NT], F32)
            neg_e = stat.tile([128, NT], F32)
            zinv = stat.tile([128, NT], F32)
            ebcast = stat.tile([128, LK], F32)
            e_nat = stat.tile([1, LK], F32)
            tT = stat.tile([128, NT, LQ], F16)
            evW1 = stat.tile([128, NT, 257], F16)

            # ---------------- stage A: sk = k @ w_k (permuted rows) -------
            for t in range(NT):
                k_tile = kv.tile([128, D], F32, tag="kv")
                nc.sync.dma_start(out=k_tile, in_=k_perm[t // 2, t % 2])
                dump = dumpp.tile([128, D], F32)
                nc.vector.scalar_tensor_tensor(
                    out=dump,
                    in0=k_tile,
                    scalar=1.0,
                    in1=wk_bc,
                    op0=ALU.bypass,
                    op1=ALU.mult,
                    accum_out=sk_sb[:, t : t + 1],
                )
            nc.scalar.activation(out=e_perm, in_=sk_sb, func=AF.Exp)
            nc.vector.tensor_scalar_mul(neg_e, e_perm, -1.0)

            # ---------------- stage E: e_nat row + ebcast ----------------
            for ch in range(4):
                ps_en = psm.tile([1, 512], F32, tag="ps")
                for cc in range(2):
                    c = 2 * ch + cc
                    for b in range(2):
                        nc.tensor.matmul(
                            ps_en[0:1, 256 * cc : 256 * (cc + 1)],
                            lhsT=e_perm[:, 2 * c + b : 2 * c + b + 1],
                            rhs=pb[:, b, :],
                            start=(b == 0),
                            stop=(b == 1),
                        )
                nc.scalar.copy(e_nat[0:1, 512 * ch : 512 * (ch + 1)], ps_en)
            for ch in range(4):
                ps_eb = psm.tile([128, 512], F32, tag="ps")
                nc.tensor.matmul(
                    ps_eb,
                    lhsT=ones_row,
                    rhs=e_nat[0:1, 512 * ch : 512 * (ch + 1)],
                    start=True,
                    stop=True,
                )
                nc.vector.tensor_copy(ebcast[:, 512 * ch : 512 * (ch + 1)], ps_eb)

            # ---------------- stage B: transposed mask -> tT (fp16) -------
            for c in range(8):
                mt = mtp.tile([128, LQ], F16)
                nc.sync.dma_start_transpose(mt, mask_f16[:, 128 * c : 128 * (c + 1)])
                mt_u8 = mt[:].bitcast(U8)  # [128, 2*LQ]
                for b in range(2):
                    t = 2 * c + b
                    nc.scalar.activation(
                        out=tT[:, t, :],
                        in_=mt_u8[:, b::2],
                        func=AF.Identity,
                        bias=e_perm[:, t : t + 1],
                        scale=neg_e[:, t : t + 1],
                    )

            # ---------------- stage C: evW1 = [e*(v@fcw.T) | 1] ----------
            for t in range(NT):
                v_tile = kv.tile([128, D], F32, tag="kv")
                nc.sync.dma_start(out=v_tile, in_=v_perm[t // 2, t % 2])
                ps_vt = psm.tile([128, 512], F32, tag="ps")
                for dh in range(2):
                    nc.tensor.transpose(
                        ps_vt[:, 128 * dh : 128 * (dh + 1)],
                        v_tile[:, 128 * dh : 128 * (dh + 1)],
                        ident,
                    )
                vT_sb = dumpp.tile([128, D], F32, tag="vT")
                nc.vector.tensor_copy(vT_sb, ps_vt[:, 0:256])
                ps_vw = psB.tile([128, 256], F32)
                for dt in range(2):
                    nc.tensor.matmul(
                        ps_vw,
                        lhsT=vT_sb[:, 128 * dt : 128 * (dt + 1)],
                        rhs=fcwT[:, dt, :],
                        start=(dt == 0),
                        stop=(dt == 1),
                    )
                nc.vector.tensor_copy(evW1[:, t, 0:256], ps_vw)
                nc.vector.memset(evW1[:, t, 256:257], 1.0)

            # ---------------- stage D: main loop over i-tiles -------------
            for it in range(NT):
                ps_num = psN.tile([128, 257], F32)
                for t in range(NT):
                    nc.tensor.matmul(
                        ps_num,
                        lhsT=tT[:, t, 128 * it : 128 * (it + 1)],
                        rhs=evW1[:, t, :],
                        start=(t == 0),
                        stop=(t == NT - 1),
                    )
                zcol = zinv[:, it : it + 1]
                nc.vector.reciprocal(zcol, ps_num[:, 256:257])

                # y = num * zinv  (attention+fc output, pre-residual)
                y_tile = small.tile([128, D], F32, tag="y")
                nc.scalar.mul(y_tile, ps_num[:, 0:256], zcol)

                # attn tile = (mask==0) * (e_j * zinv_i)
                ez = big.tile([128, LK], F32, tag="ez")
                nc.vector.tensor_scalar_mul(ez, ebcast, zcol)
                mnat = mnatp.tile([128, LK], U8)
                nc.sync.dma_start(
                    out=mnat, in_=mask_d[128 * it : 128 * (it + 1), :]
                )
                attn_t = attnp.tile([128, LK], F32)
                nc.vector.scalar_tensor_tensor(
                    out=attn_t,
                    in0=mnat,
                    scalar=0.0,
                    in1=ez,
                    op0=ALU.is_equal,
                    op1=ALU.mult,
                )
                nc.sync.dma_start(
                    out=attn_d[128 * it : 128 * (it + 1), :], in_=attn_t
                )

                # residual + layernorm
                q_tile = small.tile([128, D], F32, tag="q")
                nc.sync.dma_start(out=q_tile, in_=q_d[128 * it : 128 * (it + 1), :])
                x2 = small.tile([128, D], F32, tag="x2")
                nc.gpsimd.tensor_tensor(x2, y_tile, q_tile, ALU.add)
                stats = small.tile([128, 6], F32, tag="stats")
                nc.vector.bn_stats(stats, x2)
                mv = small.tile([128, 2], F32, tag="mv")
                nc.vector.bn_aggr(mv, stats)
                rstd = small.tile([128, 1], F32, tag="rstd")
                nc.scalar.activation(
                    out=rstd, in_=mv[:, 1:2], func=AF.Sqrt, bias=eps_sb, scale=1.0
                )
                nc.vector.reciprocal(rstd, rstd)
                nmr = small.tile([128, 1], F32, tag="nmr")
                nc.vector.tensor_scalar(
                    out=nmr,
                    in0=mv[:, 0:1],
                    scalar1=rstd,
                    scalar2=-1.0,
                    op0=ALU.mult,
                    op1=ALU.mult,
                )
                s_tile = small.tile([128, D], F32, tag="s")
                nc.scalar.activation(
                    out=s_tile, in_=x2, func=AF.Identity, scale=rstd, bias=nmr
                )
                sg = small.tile([128, D], F32, tag="sg")
                nc.gpsimd.tensor_tensor(sg, s_tile, gamma_bc, ALU.mult)
                out_t = small.tile([128, D], F32, tag="outt")
                nc.gpsimd.tensor_tensor(out_t, sg, beta_bc, ALU.add)
                nc.sync.dma_start(
                    out=out_d[128 * it : 128 * (it + 1), :], in_=out_t
                )
                if DEBUG_OUTPUTS:
                    nc.sync.dma_start(
                        out=y_dbg[128 * it : 128 * (it + 1), :], in_=y_tile
                    )
                    nc.sync.dma_start(
                        out=x2_dbg[128 * it : 128 * (it + 1), :], in_=x2
                    )

    _fix_sync_waits(nc)
    return nc


_NC = None
_LAST_IN_MAPS = None


def _get_nc():
    global _NC
    if _NC is None:
        _NC = build_nc()
    return _NC


def kernel(q, k, v, shared_attn, fc_w, ln_gamma, ln_beta, mask):
    q = np.asarray(q)
    k = np.asarray(k)
    v = np.asarray(v)
    shared_attn = np.asarray(shared_attn)
    fc_w = np.asarray(fc_w)
    ln_gamma = np.asarray(ln_gamma)
    ln_beta = np.asarray(ln_beta)
    mask_u8 = np.asarray(mask).view(np.uint8)

    wk = np.ascontiguousarray(shared_attn[:, D:])  # [1, 256]
    gamma = np.ascontiguousarray(ln_gamma.reshape(1, D))
    beta = np.ascontiguousarray(ln_beta.reshape(1, D))
    ident = np.eye(128, dtype=np.float32)
    # pb[p, b, n] = 1 if n == 2p+b
    pb = np.zeros((128, 2, 256), dtype=np.float32)
    p_idx = np.arange(128)
    for b in range(2):
        pb[p_idx, b, 2 * p_idx + b] = 1.0

    nc = _get_nc()
    in_maps = []
    for b_i in range(B):
        in_maps.append(
            {
                "q": np.ascontiguousarray(q[b_i]),
                "k": np.ascontiguousarray(k[b_i]),
                "v": np.ascontiguousarray(v[b_i]),
                "mask": np.ascontiguousarray(mask_u8[b_i]),
                "wk": wk,
                "fcw": np.ascontiguousarray(fc_w),
                "gamma": gamma,
                "beta": beta,
                "ident": ident,
                "pb": pb,
            }
        )
    global _LAST_IN_MAPS
    _LAST_IN_MAPS = in_maps
    res = run_bass_kernel_spmd(nc, in_maps, core_ids=list(range(B)))
    out = np.stack([res.results[c]["out"] for c in range(B)])
    attn = np.stack([res.results[c]["attn"] for c in range(B)])
    return out, attn


# revision 13
# speedup vs baseline: 1.0475x; 1.0475x over previous
"""Trainium2 Bass kernel for nn_Attention_4380866642117.

Math: the reference computes additive-score attention
    score[b,i,j] = q[b,i].w_q + k[b,j].w_k ; masked (mask==True -> -1e10)
    attn = softmax_j(score); out = LN(attn @ v @ fc_w.T + q)
Because the score is additive, the q-term is constant along the softmax axis
and cancels; masked logits (-1e10) underflow to exactly 0 in the f32 softmax.
Hence with e_j = exp(k[b,j].w_k):
    attn[b,i,j] = (1-m[b,i,j]) * e_j / Z_i,  Z_i = sum_j (1-m) e_j
    out_pre_ln[b,i,:] = (1/Z_i) * sum_j (1-m) e_j (v[b,j,:] @ fc_w.T) + q
so the whole attention+fc collapses to one masked matmul with fc-transformed,
e-weighted values, plus a rank-1 elementwise product for the attn output.

Distribution: data-parallel over batch (8 batches -> 8 NeuronCores).

The mask must enter the PE with j on partitions (contraction dim).  The u8
mask is loaded a second time through the DMA xbar transpose by viewing pairs
of mask bytes as one fp16 element: the transposed SBUF tile then holds rows
j=2p and j=2p+1 interleaved on partition p.  The resulting j-permutation
sigma(c,p,b) = 256c + 2p + b is absorbed by loading k and v rows with the
same permuted access pattern; the contraction is order-invariant.
"""

import numpy as np

import concourse.bass as bass
import concourse.tile as tile
from concourse import mybir
from concourse.bass_utils import run_bass_kernel_spmd

F32 = mybir.dt.float32
F16 = mybir.dt.float16
U8 = mybir.dt.uint8
AF = mybir.ActivationFunctionType
ALU = mybir.AluOpType

B, LQ, LK, D = 8, 2048, 2048, 256
DEBUG_OUTPUTS = False
NT = LK // 128  # 16 j-tiles (and i-tiles)
LN_EPS = 1e-5

# ---------------------------------------------------------------------------
# walrus in this container supports at most ONE sync wait per instruction;
# Tile emits several.  Hoist excess waits onto same-engine NoOps.
# ---------------------------------------------------------------------------


def _fix_sync_waits(nc, max_waits: int = 1) -> int:
    n = 0
    for f in nc.m.functions:
        for bb in f.blocks:
            out = []
            changed = False
            for inst in bb.instructions:
                si = inst.sync_info
                waits = list(si.on_wait) if si is not None else []
                if len(waits) > max_waits and inst.engine is not None:
                    changed = True
                    rest = waits[max_waits:]
                    for i in range(0, len(rest), max_waits):
                        nop = mybir.InstNoOp(
                            name=f"{inst.name}-syncw{n}",
                            sync_info=mybir.SyncInfo(
                                on_wait=rest[i : i + max_waits], on_update=[]
                            ),
                            bass_nofuse=True,
                            engine=inst.engine,
                        )
                        n += 1
                        out.append(nop)
                    inst.sync_info = mybir.SyncInfo(
                        on_wait=waits[:max_waits], on_update=list(si.on_update)
                    )
                out.append(inst)
            if changed:
                bb.instructions = out
    return n


def _broadcast_load(nc, sbuf_tile, dram_ap, parts=128):
    """DMA a [1, N] DRAM row replicated across `parts` partitions."""
    src = bass.AP(
        tensor=dram_ap.tensor,
        offset=dram_ap.offset,
        ap=[[0, parts]] + list(dram_ap.ap[1:]),
    )
    nc.gpsimd.dma_start(out=sbuf_tile, in_=src)


def build_nc():
    nc = bass.Bass("TRN2")

    q_d = nc.dram_tensor("q", [LQ, D], F32, kind="ExternalInput")
    k_d = nc.dram_tensor("k", [LK, D], F32, kind="ExternalInput")
    v_d = nc.dram_tensor("v", [LK, D], F32, kind="ExternalInput")
    mask_d = nc.dram_tensor("mask", [LQ, LK], U8, kind="ExternalInput")
    wk_d = nc.dram_tensor("wk", [1, D], F32, kind="ExternalInput")
    fcw_d = nc.dram_tensor("fcw", [D, D], F32, kind="ExternalInput")
    gamma_d = nc.dram_tensor("gamma", [1, D], F32, kind="ExternalInput")
    beta_d = nc.dram_tensor("beta", [1, D], F32, kind="ExternalInput")
    ident_d = nc.dram_tensor("ident", [128, 128], F32, kind="ExternalInput")
    pb_d = nc.dram_tensor("pb", [128, 2, 256], F32, kind="ExternalInput")

    out_d = nc.dram_tensor("out", [LQ, D], F32, kind="ExternalOutput")
    attn_d = nc.dram_tensor("attn", [LQ, LK], F32, kind="ExternalOutput")
    if DEBUG_OUTPUTS:
        y_dbg = nc.dram_tensor("y_dbg", [LQ, D], F32, kind="ExternalOutput")
        x2_dbg = nc.dram_tensor("x2_dbg", [LQ, D], F32, kind="ExternalOutput")

    # permuted row views: j = 256c + 2p + b  ->  tile t = 2c + b, partition p
    k_perm = k_d[:, :].rearrange("(c p two) d -> c two p d", p=128, two=2)
    v_perm = v_d[:, :].rearrange("(c p two) d -> c two p d", p=128, two=2)
    # mask viewed as fp16 pairs for the xbar transpose
    mask_f16 = mask_d[:, :].bitcast(F16)  # [2048, 1024]

    with tile.TileContext(nc) as tc:
        with (
            tc.tile_pool(name="const", bufs=1) as const,
            tc.tile_pool(name="stat", bufs=1) as stat,
            tc.tile_pool(name="kv", bufs=3) as kv,
            tc.tile_pool(name="dump", bufs=2) as dumpp,
            tc.tile_pool(name="mt", bufs=2) as mtp,
            tc.tile_pool(name="big", bufs=2) as big,
            tc.tile_pool(name="attnp", bufs=3) as attnp,
            tc.tile_pool(name="mnat", bufs=3) as mnatp,
            tc.tile_pool(name="small", bufs=3) as small,
            tc.tile_pool(name="psm", bufs=2, space="PSUM") as psm,
            tc.tile_pool(name="psB", bufs=2, space="PSUM") as psB,
            tc.tile_pool(name="psN", bufs=4, space="PSUM") as psN,
        ):
            # ---------------- constants ----------------
            ident = const.tile([128, 128], F32)
            nc.sync.dma_start(out=ident, in_=ident_d[:, :])
            pb = const.tile([128, 2, 256], F32)
            nc.sync.dma_start(out=pb, in_=pb_d[:, :, :])
            wk_bc = const.tile([128, D], F32)
            _broadcast_load(nc, wk_bc, wk_d[0:1, :])
            gamma_bc = const.tile([128, D], F32)
            _broadcast_load(nc, gamma_bc, gamma_d[0:1, :])
            beta_bc = const.tile([128, D], F32)
            _broadcast_load(nc, beta_bc, beta_d[0:1, :])
            ones_row = const.tile([1, 128], F32)
            nc.vector.memset(ones_row, 1.0)
            eps_sb = const.tile([128, 1], F32)
            nc.vector.memset(eps_sb, LN_EPS)

            # fcwT[d, c] tiles from fcw[c, d]
            fcw_sb = const.tile([128, 2, 256], F32)
            nc.sync.dma_start(
                out=fcw_sb, in_=fcw_d[:, :].rearrange("(t p) d -> p t d", p=128)
            )
            fcwT = const.tile([128, 2, 256], F32)
            for dt in range(2):
                ps_t = psm.tile([128, 512], F32, tag="ps")
                for ct in range(2):
                    nc.tensor.transpose(
                        ps_t[:, 128 * ct : 128 * (ct + 1)],
                        fcw_sb[:, ct, 128 * dt : 128 * (dt + 1)],
                        ident,
                    )
                nc.vector.tensor_copy(fcwT[:, dt, :], ps_t[:, 0:256])

            # ---------------- persistent activations ----------------
            sk_sb = stat.tile([128, NT], F32)
            e_perm = stat.tile([128, NT], F32)
            neg_e = stat.tile([128, NT], F32)
            zinv = stat.tile([128, NT], F32)
            ebcast = stat.tile([128, LK], F32)
            e_nat = stat.tile([1, LK], F32)
            tT = stat.tile([128, NT, LQ], F16)
            evW1 = stat.tile([128, NT, 257], F16)

            # ---------------- stage A: sk = k @ w_k (permuted rows) -------
            for t in range(NT):
                k_tile = kv.tile([128, D], F32, tag="kv")
                nc.sync.dma_start(out=k_tile, in_=k_perm[t // 2, t % 2])
                dump = dumpp.tile([128, D], F32)
                nc.vector.scalar_tensor_tensor(
                    out=dump,
                    in0=k_tile,
                    scalar=1.0,
                    in1=wk_bc,
                    op0=ALU.bypass,
                    op1=ALU.mult,
                    accum_out=sk_sb[:, t : t + 1],
                )
            nc.scalar.activation(out=e_perm, in_=sk_sb, func=AF.Exp)
            nc.vector.tensor_scalar_mul(neg_e, e_perm, -1.0)

            # ---------------- stage E: e_nat row + ebcast ----------------
            for ch in range(4):
                ps_en = psm.tile([1, 512], F32, tag="ps")
                for cc in range(2):
                    c = 2 * ch + cc
                    for b in range(2):
                        nc.tensor.matmul(
                            ps_en[0:1, 256 * cc : 256 * (cc + 1)],
                            lhsT=e_perm[:, 2 * c + b : 2 * c + b + 1],
                            rhs=pb[:, b, :],
                            start=(b == 0),
                            stop=(b == 1),
                        )
                nc.scalar.copy(e_nat[0:1, 512 * ch : 512 * (ch + 1)], ps_en)
            for ch in range(4):
                ps_eb = psm.tile([128, 512], F32, tag="ps")
                nc.tensor.matmul(
                    ps_eb,
                    lhsT=ones_row,
                    rhs=e_nat[0:1, 512 * ch : 512 * (ch + 1)],
                    start=True,
                    stop=True,
                )
                nc.vector.tensor_copy(ebcast[:, 512 * ch : 512 * (ch + 1)], ps_eb)

            # ---------------- stage B: transposed mask -> tT (fp16) -------
            # builds split across ACT (b==0) and DVE (b==1) to halve latency
            for c in range(8):
                mt = mtp.tile([128, LQ], F16)
                nc.scalar.dma_start_transpose(mt, mask_f16[:, 128 * c : 128 * (c + 1)])
                mt_u8 = mt[:].bitcast(U8)  # [128, 2*LQ]
                for b in range(2):
                    t = 2 * c + b
                    if b == 0:
                        nc.scalar.activation(
                            out=tT[:, t, :],
                            in_=mt_u8[:, b::2],
                            func=AF.Identity,
                            bias=e_perm[:, t : t + 1],
                            scale=neg_e[:, t : t + 1],
                        )
                    else:
                        nc.vector.tensor_scalar(
                            out=tT[:, t, :],
                            in0=mt_u8[:, b::2],
                            scalar1=neg_e[:, t : t + 1],
                            scalar2=e_perm[:, t : t + 1],
                            op0=ALU.mult,
                            op1=ALU.add,
                        )

            # ---------------- stage C: evW1 = [e*(v@fcw.T) | 1] ----------
            for t in range(NT):
                v_tile = kv.tile([128, D], F32, tag="kv")
                nc.sync.dma_start(out=v_tile, in_=v_perm[t // 2, t % 2])
                ps_vt = psm.tile([128, 512], F32, tag="ps")
                for dh in range(2):
                    nc.tensor.transpose(
                        ps_vt[:, 128 * dh : 128 * (dh + 1)],
                        v_tile[:, 128 * dh : 128 * (dh + 1)],
                        ident,
                    )
                vT_sb = dumpp.tile([128, D], F32, tag="vT")
                nc.scalar.copy(vT_sb, ps_vt[:, 0:256])
                ps_vw = psB.tile([128, 256], F32)
                for dt in range(2):
                    nc.tensor.matmul(
                        ps_vw,
                        lhsT=vT_sb[:, 128 * dt : 128 * (dt + 1)],
                        rhs=fcwT[:, dt, :],
                        start=(dt == 0),
                        stop=(dt == 1),
                    )
                nc.scalar.copy(evW1[:, t, 0:256], ps_vw)
                nc.vector.memset(evW1[:, t, 256:257], 1.0)

            # ---------------- stage D: main loop over i-tiles -------------
            # t-outer accumulation: 4 groups of 4 i-tiles, each i-tile owns a
            # PSUM bank for the whole t loop; drains pipeline against the
            # next group's matmuls.
            for g in range(4):
                ps_tiles = [
                    psN.tile([128, 257], F32, tag="num", name=f"num_{g}_{i_}")
                    for i_ in range(4)
                ]
                for t in range(NT):
                    for ii in range(4):
                        it = 4 * g + ii
                        nc.tensor.matmul(
                            ps_tiles[ii],
                            lhsT=tT[:, t, 128 * it : 128 * (it + 1)],
                            rhs=evW1[:, t, :],
                            start=(t == 0),
                            stop=(t == NT - 1),
                        )
                for ii in range(4):
                    it = 4 * g + ii
                    ps_num = ps_tiles[ii]
                    zcol = zinv[:, it : it + 1]
                    nc.vector.reciprocal(zcol, ps_num[:, 256:257])

                    # y = num * zinv  (attention+fc output, pre-residual)
                    y_tile = small.tile([128, D], F32, tag="y")
                    nc.scalar.mul(y_tile, ps_num[:, 0:256], zcol)

                    # attn tile = (mask==0) * (e_j * zinv_i)
                    ez = big.tile([128, LK], F32, tag="ez")
                    nc.vector.tensor_scalar_mul(ez, ebcast, zcol)
                    mnat = mnatp.tile([128, LK], U8)
                    nc.sync.dma_start(
                        out=mnat, in_=mask_d[128 * it : 128 * (it + 1), :]
                    )
                    attn_t = attnp.tile([128, LK], F32)
                    nc.vector.scalar_tensor_tensor(
                        out=attn_t,
                        in0=mnat,
                        scalar=0.0,
                        in1=ez,
                        op0=ALU.is_equal,
                        op1=ALU.mult,
                    )
                    nc.scalar.dma_start(
                        out=attn_d[128 * it : 128 * (it + 1), :], in_=attn_t
                    )

                    # residual + layernorm
                    q_tile = small.tile([128, D], F32, tag="q")
                    nc.sync.dma_start(
                        out=q_tile, in_=q_d[128 * it : 128 * (it + 1), :]
                    )
                    x2 = small.tile([128, D], F32, tag="x2")
                    nc.gpsimd.tensor_tensor(x2, y_tile, q_tile, ALU.add)
                    stats = small.tile([128, 6], F32, tag="stats")
                    nc.vector.bn_stats(stats, x2)
                    mv = small.tile([128, 2], F32, tag="mv")
                    nc.vector.bn_aggr(mv, stats)
                    rstd = small.tile([128, 1], F32, tag="rstd")
                    nc.scalar.activation(
                        out=rstd, in_=mv[:, 1:2], func=AF.Sqrt, bias=eps_sb, scale=1.0
                    )
                    nc.vector.reciprocal(rstd, rstd)
                    nmr = small.tile([128, 1], F32, tag="nmr")
                    nc.vector.tensor_scalar(
                        out=nmr,
                        in0=mv[:, 0:1],
                        scalar1=rstd,
                        scalar2=-1.0,
                        op0=ALU.mult,
                        op1=ALU.mult,
                    )
                    s_tile = small.tile([128, D], F32, tag="s")
                    nc.scalar.activation(
                        out=s_tile, in_=x2, func=AF.Identity, scale=rstd, bias=nmr
                    )
                    sg = small.tile([128, D], F32, tag="sg")
                    nc.gpsimd.tensor_tensor(sg, s_tile, gamma_bc, ALU.mult)
                    out_t = small.tile([128, D], F32, tag="outt")
                    nc.gpsimd.tensor_tensor(out_t, sg, beta_bc, ALU.add)
                    nc.scalar.dma_start(
                        out=out_d[128 * it : 128 * (it + 1), :], in_=out_t
                    )
                    if DEBUG_OUTPUTS:
                        nc.sync.dma_start(
                            out=y_dbg[128 * it : 128 * (it + 1), :], in_=y_tile
                        )
                        nc.sync.dma_start(
                            out=x2_dbg[128 * it : 128 * (it + 1), :], in_=x2
                        )

    _fix_sync_waits(nc)
    return nc


_NC = None
_LAST_IN_MAPS = None


def _get_nc():
    global _NC
    if _NC is None:
        _NC = build_nc()
    return _NC


def kernel(q, k, v, shared_attn, fc_w, ln_gamma, ln_beta, mask):
    q = np.asarray(q)
    k = np.asarray(k)
    v = np.asarray(v)
    shared_attn = np.asarray(shared_attn)
    fc_w = np.asarray(fc_w)
    ln_gamma = np.asarray(ln_gamma)
    ln_beta = np.asarray(ln_beta)
    mask_u8 = np.asarray(mask).view(np.uint8)

    wk = np.ascontiguousarray(shared_attn[:, D:])  # [1, 256]
    gamma = np.ascontiguousarray(ln_gamma.reshape(1, D))
    beta = np.ascontiguousarray(ln_beta.reshape(1, D))
    ident = np.eye(128, dtype=np.float32)
    # pb[p, b, n] = 1 if n == 2p+b
    pb = np.zeros((128, 2, 256), dtype=np.float32)
    p_idx = np.arange(128)
    for b in range(2):
        pb[p_idx, b, 2 * p_idx + b] = 1.0

    nc = _get_nc()
    in_maps = []
    for b_i in range(B):
        in_maps.append(
            {
                "q": np.ascontiguousarray(q[b_i]),
                "k": np.ascontiguousarray(k[b_i]),
                "v": np.ascontiguousarray(v[b_i]),
                "mask": np.ascontiguousarray(mask_u8[b_i]),
                "wk": wk,
                "fcw": np.ascontiguousarray(fc_w),
                "gamma": gamma,
                "beta": beta,
                "ident": ident,
                "pb": pb,
            }
        )
    global _LAST_IN_MAPS
    _LAST_IN_MAPS = in_maps
    res = run_bass_kernel_spmd(nc, in_maps, core_ids=list(range(B)))
    out = np.stack([res.results[c]["out"] for c in range(B)])
    attn = np.stack([res.results[c]["attn"] for c in range(B)])
    return out, attn
